# revision 27
# baseline (speedup 1.0000x reference)
"""Cross-attention (GQA + RoPE) Trainium2 Bass kernel.

Sharding: 8 cores = 4 batches x 2 head-groups.
  core i -> batch b = i // 2, head-group g = i % 2
  Each core computes 8 query heads / 2 kv heads of one batch and a
  row-parallel partial of the output projection; the host sums the two
  partials per batch.

v3 design (from the 306us baseline):
  * Zero mode switches: score matmuls contract K=128 against zero-padded
    per-head K/Q tiles (Kta/Ktb, Qta/Qtb), so every matmul on the PE runs
    in the full 128-row configuration.  The baseline's 64-row-tiled score
    pairs saved half the score stream time but cost ~270ns of mode-switch
    drain per chunk and pinned projection work to pair boundaries.
  * One 2-bank [128,1024] exp per chunk covers both heads of the pair.
  * Projection matmuls (next pair's Q proj, previous block's out proj,
    the normalization broadcast) are dripped into the chunk loop between
    the scores and the PV matmuls, covering the exp latency.
  * Prologue streams pair-0 attention chunk ranges directly behind the
    four kv DMA tiles; inputs are spread over four DMA rings so the first
    exp fires at ~8us instead of ~43us.
  * bf16 rope (PSUM result copied to bf16 once, muls at the DVE 16-bit
    rate), bf16 output partials, out slices spread over four DMA rings.
"""

import math
from collections import deque
from contextlib import ExitStack

import numpy as np
import ml_dtypes

import concourse.bass as bass
import concourse.bacc as bacc
import concourse.mybir as mybir
import concourse.tile as tile
from concourse.bass_utils import run_bass_kernel_spmd

F32 = mybir.dt.float32
R32 = mybir.dt.float32r
BF16 = mybir.dt.bfloat16

D_MODEL = 1024
N_HEADS = 16
NUM_KV_HEADS = 4
D_K = 64
ROPE_BASE = 10000.0
TQ = 2048
N_CORES = 8

NEG_BIAS = -30000.0


def _ktiles(tkv_c):
    """K-projection column tiles: two 256-wide leading tiles (so the first
    projection starts as early as possible behind the kv DMA), 512 after."""
    ks = []
    c0 = 0
    while c0 < tkv_c:
        w = min(256 if c0 < 512 else 512, tkv_c - c0)
        ks.append((c0, w))
        c0 += w
    return ks


def build_bass(tq=TQ, tkv_c=1152, t2=512):
    """Build the single-core SPMD program (same program on all 8 cores)."""
    assert t2 == 512
    nc = bacc.Bacc("TRN2", target_bir_lowering=False, debug=False)
    P = 128
    NCH = tkv_c // 128        # attention kv chunks
    NT2 = tq // t2            # tq blocks
    NPAIR = 4                 # head-pair tiles per core
    NSLICE = t2 // 128        # output rows per block
    ktiles = _ktiles(tkv_c)

    q0a = nc.dram_tensor("q0a", [P, 4, t2], BF16, kind="ExternalInput").ap()
    q0b = nc.dram_tensor("q0b", [P, 4, t2], BF16, kind="ExternalInput").ap()
    q_in = [None] + [
        nc.dram_tensor(f"q{i}", [P, 8, t2], BF16, kind="ExternalInput").ap()
        for i in range(1, NT2)
    ]
    kv_in = [
        nc.dram_tensor(f"kv_t{i}", [P, 8, w], BF16, kind="ExternalInput").ap()
        for i, (_c0, w) in enumerate(ktiles)
    ]
    wq0 = nc.dram_tensor("wq0", [P, 8, 128], BF16, kind="ExternalInput").ap()
    wqr = nc.dram_tensor("wqr", [P, 8, 384], BF16, kind="ExternalInput").ap()
    wkv = nc.dram_tensor("wkv", [P, 8, 256], BF16, kind="ExternalInput").ap()
    wout = nc.dram_tensor("wout", [P, 4, D_MODEL], BF16, kind="ExternalInput").ap()
    csK = nc.dram_tensor("csK", [P, 2, tkv_c], BF16, kind="ExternalInput").ap()
    csQ0 = nc.dram_tensor("csQ0", [P, 2, t2], BF16, kind="ExternalInput").ap()
    csQr = nc.dram_tensor("csQr", [P, 2, tq - t2], BF16, kind="ExternalInput").ap()
    maskb = nc.dram_tensor("maskb", [P, NCH], F32, kind="ExternalInput").ap()
    e2 = nc.dram_tensor("e2", [P, P], R32, kind="ExternalInput").ap()
    out = nc.dram_tensor("out", [tq, D_MODEL], BF16, kind="ExternalOutput").ap()

    with tile.TileContext(nc) as tc, ExitStack() as ctx:
        const = ctx.enter_context(tc.tile_pool(name="const", bufs=1))
        qpool = ctx.enter_context(tc.tile_pool(name="qpool", bufs=1))
        apool = ctx.enter_context(tc.tile_pool(name="apool", bufs=1))
        workp = ctx.enter_context(tc.tile_pool(name="workp", bufs=3))
        ropep = ctx.enter_context(tc.tile_pool(name="ropep", bufs=2))
        # PSUM: scores pool 2 x [128,1024] (4 banks), acc pool 2 x [65,512]
        # (2 banks), misc pool 2 x [128,512] (2 banks) -> 8 banks total.
        pp_sc = ctx.enter_context(tc.tile_pool(name="pp_sc", bufs=2, space="PSUM"))
        pp_acc = ctx.enter_context(tc.tile_pool(name="pp_acc", bufs=2, space="PSUM"))
        pp_big = ctx.enter_context(tc.tile_pool(name="pp_big", bufs=2, space="PSUM"))

        def MM(out_ap, lhsT, rhs, start, stop, chain=None):
            inst = nc.tensor.matmul(out_ap, lhsT, rhs, start=start, stop=stop)
            if chain is not None:
                tc.chain_iter_dep(chain, inst.ins)
            return inst

        def chain_dve(inst):
            tc.chain_iter_dep("dve_norm", inst.ins)
            return inst

        # ---- DMA triggers first so transfers start flowing ----------------
        # Only the scalar and sync rings are hardware-DGE (~190 GB/s); the
        # gpsimd ring is software-DGE (~30 GB/s) and is not used at all.
        # The K side rides the scalar ring, the Q side the sync ring, with
        # pair-0-critical pieces split out and first.
        # scalar ring: K side
        wkv_sb = const.tile([P, 8, 256], BF16)
        nc.scalar.dma_start(out=wkv_sb, in_=wkv)
        kv_sb = [
            const.tile([P, 8, w], BF16, name=f"kvt{i}")
            for i, (_c0, w) in enumerate(ktiles)
        ]
        nc.scalar.dma_start(out=kv_sb[0], in_=kv_in[0])
        csK_sb = const.tile([P, 2, tkv_c], BF16)
        nc.scalar.dma_start(out=csK_sb, in_=csK)
        mask_sb = const.tile([P, NCH], F32)
        nc.scalar.dma_start(out=mask_sb, in_=maskb)
        nc.scalar.dma_start(out=kv_sb[1], in_=kv_in[1])
        nc.scalar.dma_start(out=kv_sb[3], in_=kv_in[3])
        wk_sb = wkv_sb[:, :, 0:128]
        wv_sb = wkv_sb[:, :, 128:256]
        cosK_sb = csK_sb[:, 0, :]
        sinK_sb = csK_sb[:, 1, :]

        # sync ring: Q side, then late bulk
        wq0_sb = const.tile([P, 8, 128], BF16)
        nc.sync.dma_start(out=wq0_sb, in_=wq0)
        q0_sb = const.tile([P, 8, t2], BF16, name="qsb0")
        nc.sync.dma_start(out=q0_sb[:, 0:4, :], in_=q0a)
        nc.sync.dma_start(out=q0_sb[:, 4:8, :], in_=q0b)
        csQ0_sb = const.tile([P, 2, t2], BF16)
        nc.sync.dma_start(out=csQ0_sb, in_=csQ0)
        nc.sync.dma_start(out=kv_sb[2], in_=kv_in[2])
        wqr_sb = const.tile([P, 8, 384], BF16)
        nc.sync.dma_start(out=wqr_sb, in_=wqr)
        csQr_sb = const.tile([P, 2, tq - t2], BF16)
        nc.sync.dma_start(out=csQr_sb, in_=csQr)
        q_sb = [q0_sb] + [
            const.tile([P, 8, t2], BF16, name=f"qsb{i}") for i in range(1, NT2)
        ]
        nc.sync.dma_start(out=q_sb[1], in_=q_in[1])
        e2_sb = const.tile([P, P], R32)
        nc.sync.dma_start(out=e2_sb, in_=e2)
        nc.sync.dma_start(out=q_sb[2], in_=q_in[2])
        nc.sync.dma_start(out=q_sb[3], in_=q_in[3])
        wout_sb = const.tile([P, 4, D_MODEL], BF16)
        nc.sync.dma_start(out=wout_sb, in_=wout)

        def wq_ap(j, d):
            if j == 0:
                return wq0_sb[:, d, :]
            return wqr_sb[:, d, (j - 1) * 128 : j * 128]

        def cosQ_ap(b0, p_lo, p_hi, w):
            # cos rows [p_lo:p_hi], q cols [b0:b0+w]
            if b0 + w <= t2:
                return csQ0_sb[:, 0, b0 : b0 + w][p_lo:p_hi]
            return csQr_sb[:, 0, b0 - t2 : b0 - t2 + w][p_lo:p_hi]

        def sinQ_ap(b0, p_lo, p_hi, w):
            if b0 + w <= t2:
                return csQ0_sb[:, 1, b0 : b0 + w][p_lo:p_hi]
            return csQr_sb[:, 1, b0 - t2 : b0 - t2 + w][p_lo:p_hi]

        # ---- constants set up on-engine (no DMA) --------------------------
        # Critical-path zero/one fills go on the otherwise-idle Vector
        # engine so they complete before the first scores matmul (~7us);
        # late-needed ones ride the GpSimd engine behind its DMA triggers.
        Vt = [const.tile([P, NCH * 65], BF16, name=f"Vt{i}") for i in range(2)]
        for i in range(2):
            nc.vector.memset(
                Vt[i].rearrange("p (c k) -> p c k", k=65)[:, :, 64], 1.0
            )  # critical: first PV reads the ones column at ~20us
        # zero-padded per-head K tiles: Kta rows 64:128 = 0, Ktb rows 0:64 = 0
        Kta = const.tile([P, tkv_c], BF16)
        Ktb = const.tile([P, tkv_c], BF16)
        nc.vector.memset(Kta[64:128, :], 0.0)
        nc.vector.memset(Ktb[0:64, :], 0.0)
        # inv broadcast staging: head0 inv in row 0, head1 inv in row 32,
        # all other rows memset to a safe finite value (multiplied by e2=0).
        invp_tiles = [const.tile([P, t2], R32, name=f"invp{i}") for i in range(4)]
        nc.vector.memset(invp_tiles[0].bitcast(F32), 1.0)
        for tl in invp_tiles[1:]:
            nc.gpsimd.memset(tl.bitcast(F32), 1.0)

        # zero-padded per-head Q tiles, double generation
        Qt = [
            [
                (
                    qpool.tile([P, t2], BF16, tag=f"Qa{j}g{g}", name=f"Qta{j}g{g}"),
                    qpool.tile([P, t2], BF16, tag=f"Qb{j}g{g}", name=f"Qtb{j}g{g}"),
                )
                for j in range(NPAIR)
            ]
            for g in range(2)
        ]
        for g in range(2):
            for j in range(NPAIR):
                eng = nc.vector if (g == 0 and j == 0) else nc.gpsimd
                eng.memset(Qt[g][j][0][64:128, :], 0.0)
                eng.memset(Qt[g][j][1][0:64, :], 0.0)
        At = [
            [
                apool.tile([P, t2], BF16, tag=f"A{j}g{g}", name=f"At{j}g{g}")
                for j in range(NPAIR)
            ]
            for g in range(2)
        ]

        rope_flip = [0]

        def rope_apply(dests, ps, col0, width, cos_fn, sin_fn):
            """dests = (dest_a rows 0:64, dest_b rows 64:128) bf16 SBUF =
            rope(ps[128, width] PSUM f32), positions col0..col0+width.
            Rows are two stacked heads, each [x1(32); x2(32)].  The PSUM
            result is first copied to bf16 so the rope muls run at the DVE
            16-bit rate; the copy engine alternates Scalar/Vector."""
            qb = ropep.tile([P, t2], BF16, tag="ropeq", name="qb")
            qb_ = qb[:, :width]
            with nc.allow_low_precision("rope bf16"):
                if rope_flip[0] % 2 == 0:
                    nc.scalar.copy(out=qb_, in_=ps)
                else:
                    nc.vector.tensor_copy(out=qb_, in_=ps)
            rope_flip[0] += 1
            t_cos = ropep.tile([P, t2], BF16, tag="rope", name="t_cos")
            t_u = ropep.tile([P, t2], BF16, tag="rope", name="t_u")
            tc_ = t_cos[:, :width]
            tu_ = t_u[:, :width]
            nc.vector.tensor_mul(tc_, qb_, cos_fn(col0, 0, 128, width))
            # sin rows carry the sign for their SOURCE row (+s for x1 rows,
            # -s for x2 rows) so both tensor_mul inputs share a base
            # partition; only the output is partition-shifted by +-32.
            for b0 in (0, 64):
                nc.vector.tensor_mul(
                    tu_[b0 : b0 + 32, :],
                    qb_[b0 + 32 : b0 + 64, :],
                    sin_fn(col0, b0 + 32, b0 + 64, width),
                )
                nc.vector.tensor_mul(
                    tu_[b0 + 32 : b0 + 64, :],
                    qb_[b0 : b0 + 32, :],
                    sin_fn(col0, b0, b0 + 32, width),
                )
            da, db = dests
            nc.vector.tensor_add(da[0:64, :width], tc_[0:64, :], tu_[0:64, :])
            nc.vector.tensor_add(
                db[64:128, :width], tc_[64:128, :], tu_[64:128, :]
            )

        # ---- filler tasks: dripped into the chunk loop --------------------
        fillers = deque()

        def drip(n=1):
            k = 0
            while fillers and k < n:
                fillers.popleft()()
                k += 1

        def drain_fillers():
            while fillers:
                fillers.popleft()()

        # ---- phase KV: K/V projections ------------------------------------
        def kv_tile(kt):
            kc0, kw = ktiles[kt]
            ps_k = pp_big.tile([P, 512], F32, tag="big", name="ps_k")
            pk = ps_k[:, :kw]
            for d in range(8):
                MM(pk, wk_sb[:, d, :], kv_sb[kt][:, d, :], d == 0, d == 7)
            rope_apply(
                (Kta[:, kc0 : kc0 + kw], Ktb[:, kc0 : kc0 + kw]),
                pk, kc0, kw,
                lambda c0, pl, ph, w: cosK_sb[pl:ph, c0 : c0 + w],
                lambda c0, pl, ph, w: sinK_sb[pl:ph, c0 : c0 + w],
            )
            for s in range(kw // 128):
                ps_v = pp_big.tile([P, 512], F32, tag="big", name="ps_v")
                pv = ps_v[:, 0:128]
                lv = slice(s * 128, (s + 1) * 128)
                for d in range(8):
                    MM(pv, kv_sb[kt][:, d, lv], wv_sb[:, d, :], d == 0, d == 7)
                c = kc0 // 128 + s
                with nc.allow_low_precision("V bf16"):
                    nc.vector.tensor_copy(
                        out=Vt[0][:, c * 65 : c * 65 + 64], in_=pv[:, 0:64]
                    )
                    nc.vector.tensor_copy(
                        out=Vt[1][:, c * 65 : c * 65 + 64], in_=pv[:, 64:128]
                    )

        pending = []
        pair_seq = [0]

        def queue_qproj(it2, j):
            """Queue the Q projection for pair (it2, j) as drip tasks:
            two 4-matmul tasks plus a rope task."""
            st = {}

            def mk_mms(d_lo, d_hi):
                def t():
                    if "ps" not in st:
                        st["ps"] = pp_big.tile([P, t2], F32, tag="big", name="ps_q")
                    for d in range(d_lo, d_hi):
                        MM(
                            st["ps"],
                            wq_ap(j, d),
                            q_sb[it2][:, d, :],
                            d == 0,
                            d == 7,
                            chain="pe_attn",
                        )
                return t

            def t_rope():
                rope_apply(
                    Qt[it2 % 2][j], st["ps"], it2 * t2, t2,
                    lambda c0, pl, ph, w: cosQ_ap(c0, pl, ph, w),
                    lambda c0, pl, ph, w: sinQ_ap(c0, pl, ph, w),
                )

            fillers.append(mk_mms(0, 4))
            fillers.append(mk_mms(4, 8))
            fillers.append(t_rope)

        def flush_norm():
            if not pending:
                return
            U0, U1, invp, j_, attn_cur = pending.pop(0)
            Us = (U0, U1)
            ps_b = pp_big.tile([P, 512], F32, tag="big", name="ps_b")
            MM(ps_b, e2_sb, invp, True, True, chain="pe_attn")
            for ab, base in ((0, 0), (1, 64)):
                with nc.allow_low_precision("attnT bf16"):
                    chain_dve(
                        nc.vector.tensor_mul(
                            attn_cur[j_][base : base + 64, :],
                            Us[ab][0:64, :],
                            ps_b[base : base + 64, :],
                        )
                    )

        out_dma_seq = [0]

        def queue_outproj(it2, s):
            """Queue output-projection slice s of block it2 as two drip
            tasks (4 matmuls + evacuation each); the DMA rides the 2nd."""
            attn_cur = At[it2 % 2]
            st = {}

            def mk_half(n):
                def t():
                    if "ob" not in st:
                        st["ob"] = ropep.tile(
                            [P, D_MODEL], BF16, tag="ob", name="ob", bufs=2
                        )
                    ps_f = pp_big.tile([P, 512], F32, tag="big", name="ps_f")
                    for p_ in range(NPAIR):
                        MM(
                            ps_f,
                            attn_cur[p_][:, s * 128 : (s + 1) * 128],
                            wout_sb[:, p_, n * 512 : (n + 1) * 512],
                            p_ == 0,
                            p_ == NPAIR - 1,
                            chain="pe_attn",
                        )
                    with nc.allow_low_precision("out bf16"):
                        nc.scalar.copy(
                            out=st["ob"][:, n * 512 : n * 512 + 256],
                            in_=ps_f[:, 0:256],
                        )
                        nc.vector.tensor_copy(
                            out=st["ob"][:, n * 512 + 256 : (n + 1) * 512],
                            in_=ps_f[:, 256:512],
                        )
                    if n == 1:
                        r0 = it2 * t2 + s * 128
                        out_dma_seq[0] += 1
                        nc.sync.dma_start(out=out[r0 : r0 + 128, :], in_=st["ob"])
                return t

            fillers.append(mk_half(0))
            fillers.append(mk_half(1))

        def make_pair(it2, j):
            """Resumable attention for head-pair j of block it2: run(c_lo,
            c_hi) emits chunk work; finish() emits the PV tail and the
            normalization prologue (U/den copies, reciprocal, inv pack)."""
            Qta_, Qtb_ = Qt[it2 % 2][j]
            attn_cur = At[it2 % 2]
            ps_os = [
                pp_acc.tile([65, t2], F32, tag="acc", name=f"ps_o{ab}")
                for ab in range(2)
            ]
            st = {"q": deque()}

            def emit_pv(c_, ex2_):
                for ab in range(2):
                    MM(
                        ps_os[ab][:, :],
                        Vt[ab][:, c_ * 65 : c_ * 65 + 65],
                        ex2_[:, ab * 512 : (ab + 1) * 512],
                        c_ == 0,
                        c_ == NCH - 1,
                        chain="pe_attn",
                    )

            def run(c_lo, c_hi):
                # Issue order per chunk: scores pair -> filler -> the PV
                # pair from TWO chunks back, whose exp is guaranteed done
                # (lag-2 keeps the in-order PE queue free of exp waits).
                for c in range(c_lo, c_hi):
                    ps_s2 = pp_sc.tile([P, 1024], F32, tag="sc", name="ps_s2")
                    for ab, KtX, QtX in ((0, Kta, Qta_), (1, Ktb, Qtb_)):
                        MM(
                            ps_s2[:, ab * 512 : (ab + 1) * 512],
                            KtX[:, c * 128 : (c + 1) * 128],
                            QtX,
                            True,
                            True,
                            chain="pe_attn",
                        )
                    ex2 = workp.tile([P, 1024], BF16, tag="expT", name="ex2", bufs=5)
                    nc.scalar.activation(
                        out=ex2,
                        in_=ps_s2,
                        func=mybir.ActivationFunctionType.Exp,
                        bias=mask_sb[:, c : c + 1],
                        scale=0.125,
                    )
                    st["q"].append((c, ex2))
                    drip(1)
                    if len(st["q"]) > 2:
                        cp, exp_ = st["q"].popleft()
                        emit_pv(cp, exp_)

            def finish():
                while st["q"]:
                    cp, exp_ = st["q"].popleft()
                    emit_pv(cp, exp_)
                # flush the previous pair first: its bcast matmul runs now
                # (reciprocal long done), and its muls free ps_b slots early.
                while pending:
                    flush_norm()
                # accumulator copies BEFORE the reciprocal: the in-order DVE
                # must release both PSUM slots promptly.
                invp = invp_tiles[pair_seq[0] % 4]
                pair_seq[0] += 1
                # U copies first (and split across Scalar/Vector) so both
                # accumulator banks release as fast as possible -- the next
                # pair's first PV waits on them.  Row 64 carries the den.
                Us = []
                for ab in range(2):
                    U = workp.tile([65, t2], F32, tag="unorm", name="U", bufs=4)
                    nc.scalar.copy(out=U, in_=ps_os[ab])
                    Us.append(U)
                for ab in range(2):
                    den = workp.tile([1, t2], F32, tag="den", name="den", bufs=4)
                    chain_dve(
                        nc.vector.tensor_copy(out=den, in_=Us[ab][64:65, :])
                    )
                    inv_f = workp.tile([1, t2], F32, tag="invf", name="inv_f", bufs=4)
                    chain_dve(nc.vector.reciprocal_approx_fast(out=inv_f, in_=den))
                    with nc.allow_low_precision("f32r softmax denom"):
                        chain_dve(
                            nc.vector.tensor_copy(
                                out=invp[32 * ab : 32 * ab + 1, :], in_=inv_f
                            )
                        )
                pending.append((Us[0], Us[1], invp, j, attn_cur))

            return run, finish

        # ---- pipeline -----------------------------------------------------
        # chunk ranges covered by each kv tile
        ranges = []
        acc0 = 0
        for _kc0, kw in ktiles:
            ranges.append((acc0, acc0 + kw // 128))
            acc0 += kw // 128

        # prologue: pair (0,0) streams directly behind the kv tiles;
        # its Q projection is emitted first since the q0/wq0 DMAs land
        # before kv0 does.
        queue_qproj(0, 0)
        drain_fillers()
        kv_tile(0)
        r0, f0 = make_pair(0, 0)
        for kt in range(len(ktiles)):
            if kt > 0:
                kv_tile(kt)
            if kt == 2:
                queue_qproj(0, 1)
            r0(*ranges[kt])
        f0()
        queue_qproj(0, 2)

        # steady state: pairs (0,1) .. (3,3).  Work for the NEXT pair's
        # drip budget is queued only AFTER fin(), which first emits the
        # pending normalization flush of the PREVIOUS pair -- an out-proj
        # slice queued any earlier would read a stale attn tile.
        plist = [(b2, jj) for b2 in range(NT2) for jj in range(NPAIR)][1:]
        for idx, (it2, j) in enumerate(plist):
            run, fin = make_pair(it2, j)
            run(0, NCH)
            fin()
            la = it2 * NPAIR + j + 2
            if la < NT2 * NPAIR:
                queue_qproj(la // NPAIR, la % NPAIR)
            if it2 > 0:
                queue_outproj(it2 - 1, j)
        drain_fillers()
        while pending:
            flush_norm()
        for s in range(NSLICE):
            queue_outproj(NT2 - 1, s)
        drain_fillers()

    nc.compile()
    return nc


# ---------------------------------------------------------------------------
# host-side sharding / prep
# ---------------------------------------------------------------------------

_HEAD_PERM = [0, 4, 1, 5, 2, 6, 3, 7]  # local head order inside pair tiles


def _rope_tables(positions):
    """cos/sin tables [128, len(positions)].  The sin sign is baked in per
    SOURCE row: x1 rows (first 32 of each head) carry +s, x2 rows carry -s,
    matching the partition-aligned rope muls in the kernel."""
    theta = ROPE_BASE ** (-np.arange(0, D_K, 2, dtype=np.float64) / D_K)  # [32]
    ang = positions.astype(np.float64)[:, None] * theta[None, :]  # [T,32]
    c = np.cos(ang).T.astype(np.float32)  # [32, T]
    s = np.sin(ang).T.astype(np.float32)
    cosF = np.concatenate([c, c, c, c], axis=0)
    sinF = np.concatenate([s, -s, s, -s], axis=0)
    return np.ascontiguousarray(cosF), np.ascontiguousarray(sinF)


def _pack8(a, c, width):
    """[c*128, width] -> [128, c, width] with row d = c_idx*128 + p."""
    return np.ascontiguousarray(a.reshape(c, 128, width).transpose(1, 0, 2))


def make_in_maps(query, key_value, kv_mask, w_q, w_k, w_v, w_out, tq=TQ):
    nb = query.shape[0]
    bf = ml_dtypes.bfloat16

    idxs = [np.nonzero(kv_mask[b])[0] for b in range(nb)]
    nmax = max((len(i) for i in idxs), default=1)
    tkv_c = max(256, int(math.ceil(max(nmax, 1) / 128.0)) * 128)
    nch = tkv_c // 128

    cosQ, sinQ = _rope_tables(np.arange(tq))
    cosQ_bf = cosQ.astype(bf)
    sinQ_bf = sinQ.astype(bf)
    e2 = np.zeros((128, 128), np.float32)
    e2[0, 0:64] = 1.0
    e2[32, 64:128] = 1.0

    col_perm = np.concatenate(
        [np.arange(h * D_K, (h + 1) * D_K) for h in _HEAD_PERM]
    )
    in_maps = []
    for core in range(2 * nb):
        b = core // 2
        g = core % 2
        idx = idxs[b]
        nv = len(idx)

        kv_c = np.zeros((tkv_c, D_MODEL), np.float32)
        kv_c[:nv] = key_value[b][idx]
        kvT = np.ascontiguousarray(kv_c.T)  # [1024, tkv_c]

        pos = np.zeros(tkv_c, np.int64)
        pos[:nv] = idx
        cosK, sinK = _rope_tables(pos)

        maskb = np.full(tkv_c, NEG_BIAS, np.float32)
        maskb[:nv] = 0.0
        maskb = np.ascontiguousarray(maskb.reshape(nch, 128).T)

        qT = np.ascontiguousarray(query[b].T)  # [1024, tq]

        wq_g = w_q[:, g * 512 : (g + 1) * 512][:, col_perm]
        wk_g = w_k[:, g * 128 : (g + 1) * 128]
        wv_g = w_v[:, g * 128 : (g + 1) * 128]
        wout_g = w_out[g * 512 : (g + 1) * 512, :][col_perm, :]

        wkv_g = np.concatenate([wk_g, wv_g], axis=1)  # [1024, 256]
        wq_p = _pack8(np.ascontiguousarray(wq_g), 8, 512).astype(bf)
        csQ_p = np.ascontiguousarray(np.stack([cosQ, sinQ], axis=1)).astype(bf)
        m = {
            "wq0": np.ascontiguousarray(wq_p[:, :, 0:128]),
            "wqr": np.ascontiguousarray(wq_p[:, :, 128:512]),
            "wkv": _pack8(np.ascontiguousarray(wkv_g), 8, 256).astype(bf),
            "wout": _pack8(np.ascontiguousarray(wout_g), 4, D_MODEL).astype(bf),
            "csK": np.ascontiguousarray(
                np.stack([cosK, sinK], axis=1)
            ).astype(bf),
            "csQ0": np.ascontiguousarray(csQ_p[:, :, 0:512]),
            "csQr": np.ascontiguousarray(csQ_p[:, :, 512:]),
            "maskb": maskb,
            "e2": e2,
        }
        for i, (c0, w) in enumerate(_ktiles(tkv_c)):
            m[f"kv_t{i}"] = _pack8(
                np.ascontiguousarray(kvT[:, c0 : c0 + w]), 8, w
            ).astype(bf)
        q0_p = _pack8(np.ascontiguousarray(qT[:, 0:512]), 8, 512).astype(bf)
        m["q0a"] = np.ascontiguousarray(q0_p[:, 0:4, :])
        m["q0b"] = np.ascontiguousarray(q0_p[:, 4:8, :])
        for i in range(1, tq // 512):
            m[f"q{i}"] = _pack8(
                np.ascontiguousarray(qT[:, i * 512 : (i + 1) * 512]), 8, 512
            ).astype(bf)
        in_maps.append(m)
    return in_maps, tkv_c


_NC_CACHE = {}


T2 = 512


def _get_nc(tq, tkv_c):
    key = (tq, tkv_c, T2)
    if key not in _NC_CACHE:
        _NC_CACHE[key] = build_bass(tq, tkv_c, T2)
    return _NC_CACHE[key]


def _run(inputs, trace=False):
    query = np.asarray(inputs["query"], dtype=np.float32)
    key_value = np.asarray(inputs["key_value"], dtype=np.float32)
    kv_mask = np.asarray(inputs["kv_mask"])
    w_q = np.asarray(inputs["w_q"], dtype=np.float32)
    w_k = np.asarray(inputs["w_k"], dtype=np.float32)
    w_v = np.asarray(inputs["w_v"], dtype=np.float32)
    w_out = np.asarray(inputs["w_out"], dtype=np.float32)
    nb, tq, _ = query.shape

    in_maps, tkv_c = make_in_maps(query, key_value, kv_mask, w_q, w_k, w_v, w_out, tq)
    nc = _get_nc(tq, tkv_c)
    res = run_bass_kernel_spmd(
        nc, in_maps, list(range(2 * nb)), trace=trace, trace_cores=[0]
    )
    outs = [np.asarray(r["out"]).astype(np.float32) for r in res.results]
    full = np.stack([outs[2 * b] + outs[2 * b + 1] for b in range(nb)])

    query_mask = np.asarray(inputs["query_mask"])
    if not query_mask.all():
        # masked query rows: reference yields uniform attention over all kv
        for b in range(nb):
            rows = ~query_mask[b]
            if rows.any():
                V = key_value[b] @ w_v  # [tkv, 256]
                meanV = V.mean(axis=0)  # [256]
                group = N_HEADS // NUM_KV_HEADS
                feat = np.concatenate([meanV.reshape(NUM_KV_HEADS, D_K)[h // group]
                                       for h in range(N_HEADS)])
                full[b, rows, :] = feat @ w_out
    return full.astype(np.float32), res


def kernel(**inputs):
    out, _ = _run(inputs, trace=False)
    return out


def kernel_traced(**inputs):
    out, res = _run(inputs, trace=True)
    return out, res


if __name__ == "__main__":
    print("kernel.py is a library; use test.py")
